# revision 20
# baseline (speedup 1.0000x reference)
"""Trainium2 Bass kernel for nn_LLPKTMultiType (LLPKT knowledge tracing).

Strategy: pure data parallel, 4 samples per core x 8 cores.

The 850-step sequential erase-add memory recurrence
    M_{s+1}[c,d] = M_s[c,d] * (1 - w_s[c] e_s[d]) + w_s[c] a_s[d]
is evaluated with the DVE TensorTensorScan instruction (state = d0*state + d1
along the free axis, fp32 internal state).  Layout: partitions =
(2 samples x 64 d), free = (50 concepts x step positions) c-major so every
scan-phase op is contiguous.  w is transposed on-chip (PE) and stored
c-major in DRAM, then broadcast-DMA'd across the 64 d-partitions in fp16.
Carry across chunks rides a zero-multiplier reset column per concept
segment, injected by the Pool engine (strided writes are pathologically
slow on DVE but cheap on Pool).  Readouts sum_c w[c] M_s[c,d] use DVE
scalar_tensor_tensor with fp32 accum_out.
"""

import os
import sys

import numpy as np

sys.path.insert(0, "/opt/trn_rl_repo")

B, S, L = 32, 50, 16
C, D = 50, 64
NQ, NL, NU = 10000, 2000, 5000
QV = NQ + NL + 1          # 12001
QAV = 2 * NQ + 1          # 20001
H4 = 4 * D                # 256
EPS = 1e-5

BL = 4                    # samples per core
NCORES = 8
NS = S * (L + 1)          # 850 flat update steps per sample
TC = 5                    # outer timesteps per scan chunk
SC = S // TC              # 10 chunks
CH = 17 * TC              # 85 update positions per chunk
LROWS = S * L             # 800 real lecture rows per sample
LPAD = 896                # padded to 7*128
QPAD = 128

_BUILT = None


def _build():
    import concourse.bass as bass
    import concourse.bacc as bacc
    import concourse.mybir as mybir
    import concourse.tile as tile
    from concourse.masks import make_identity

    f32 = mybir.dt.float32
    f16 = mybir.dt.float16
    i32 = mybir.dt.int32
    AX = mybir.AxisListType
    OP = mybir.AluOpType
    AF = mybir.ActivationFunctionType

    nc = bacc.Bacc("TRN2", target_bir_lowering=False, debug=False,
                   num_devices=NCORES)

    din = lambda n, sh, dt=f32: nc.dram_tensor(n, sh, dt, kind="ExternalInput").ap()
    idx_all = din("idx_all", [BL, 10, QPAD], i32)
    q_embed = din("q_embed", [QV, D])
    qa_embed = din("qa_embed", [QAV, D])
    key = din("key", [C, D])
    M0 = din("M0", [C, D])
    W_ea = din("W_ea", [D, 2 * D])               # W_e | W_a packed
    W0 = din("W0", [H4, H4])
    W1 = din("W1", [H4, H4])
    Wout = din("Wout", [H4])
    biases = din("biases", [2 * H4 + 2 * D])     # b0 | b1 | b_e | b_a
    gamma_beta = din("gamma_beta", [2 * H4])
    b_out = din("b_out", [1])
    preds = nc.dram_tensor("preds", [BL, S], f32, kind="ExternalOutput").ap()

    with tile.TileContext(nc) as tc:
        with (
            tc.tile_pool(name="persist", bufs=1) as pp,
            tc.tile_pool(name="xt", bufs=2) as xp,
            tc.tile_pool(name="gs", bufs=2) as gsp,
            tc.tile_pool(name="gather", bufs=6) as gp,
            tc.tile_pool(name="graws", bufs=1) as grp,
            tc.tile_pool(name="cw", bufs=4) as cw,
            tc.tile_pool(name="cuv", bufs=3) as cuv,
            tc.tile_pool(name="cm", bufs=2) as cm,
            tc.tile_pool(name="psum", bufs=6, space="PSUM") as psp,
            tc.tile_pool(name="psum2", bufs=2, space="PSUM") as psp2,
            tc.tile_pool(name="dram", bufs=1, space="DRAM") as dp,
        ):
            # ---------------- constants / weights ----------------
            ident = pp.tile([128, 128], f32, tag="ident")
            make_identity(nc, ident[:])

            KT = pp.tile([D, C], f32, tag="KT")                    # [d, c]
            nc.sync.dma_start(KT[:], key.rearrange("c d -> d c"))
            Wea_sb = pp.tile([D, 2 * D], f32, tag="Wea")
            nc.sync.dma_start(Wea_sb[:], W_ea)
            W01 = pp.tile([128, 4, H4], f32, tag="W01")
            Wout_rep = pp.tile([128, H4], f32, tag="Woutr")
            bias_rep = pp.tile([128, 2 * H4 + 2 * D], f32, tag="biasr")
            gb_rep = pp.tile([S, 2 * H4], f32, tag="gbr")
            bout_rep = pp.tile([128, 1], f32, tag="boutr")
            M0Tf = pp.tile([128, C], f32, tag="M0Tf")
            M0T = pp.tile([128, C], f16, tag="M0T")
            # per-partition gate biases for sigma/tanh applied at psum drain:
            # rows 0:D = b_e (erase), rows D:2D = b_a (add)
            bias_ea = pp.tile([128, 1], f32, tag="biasea")
            nc.sync.dma_start(bias_ea[0:D, :], biases[2 * H4:2 * H4 + D][:, None])
            nc.sync.dma_start(bias_ea[D:2 * D, :],
                              biases[2 * H4 + D:2 * H4 + 2 * D][:, None])

            E = [pp.tile([128, NS], f16, tag=f"E{p}", name=f"E{p}") for p in range(2)]
            A = [pp.tile([128, NS], f16, tag=f"A{p}", name=f"A{p}") for p in range(2)]
            lr = [pp.tile([128, S], f32, tag=f"lr{p}", name=f"lr{p}") for p in range(2)]
            qr = [pp.tile([128, S], f32, tag=f"qr{p}", name=f"qr{p}") for p in range(2)]
            q_raw = [pp.tile([128, D], f32, tag=f"qraw{b}", name=f"qraw{b}") for b in range(BL)]
            le_raw = [pp.tile([128, D], f32, tag=f"leraw{b}", name=f"leraw{b}") for b in range(BL)]
            wT_sb = [pp.tile([C, NS], f16, tag=f"wT{b}", name=f"wT{b}") for b in range(BL)]
            # w stored per sample as [chunk, c, s] so each broadcast
            # descriptor is one contiguous 8.5KB burst per partition
            w_dram = [dp.tile([SC, C, CH], f16,
                              tag=f"wdram{b}", name=f"wdram{b}")
                      for b in range(BL)]

            def psum_t():
                return psp.tile([128, 128], f32, space="PSUM", tag="tp",
                                name="tp")

            # ---------------- gather + dense phase ----------------
            def load_idx(b):
                it = gp.tile([128, 10], i32, tag="idx")
                nc.sync.dma_start(it[:], idx_all[b].rearrange("j r -> r j"))
                return it

            def gather_chunk(it, j, table, dst=None):
                g = dst if dst is not None else gp.tile([128, D], f32, tag="graw",
                                                        name="graw")
                nc.gpsimd.indirect_dma_start(
                    out=g[:], out_offset=None, in_=table,
                    in_offset=bass.IndirectOffsetOnAxis(ap=it[:, j:j + 1], axis=0))
                return g

            def xT_of(g, tag):
                ps = psum_t()[0:D, :]
                nc.tensor.transpose(out=ps, in_=g[:], identity=ident[:])
                xT = xp.tile([D, 128], f32, tag=tag)
                nc.vector.tensor_copy(xT[:], ps)
                return xT

            def corr_T(xT):
                """softmax(x @ K^T) transposed -> [C, 128] psum (f32).

                Logits are tiny (|x| ~ .1) so the max-subtraction is skipped.
                """
                psc = psum_t()[:, 0:C]
                nc.tensor.matmul(psc, lhsT=xT[:], rhs=KT[:],
                                 start=True, stop=True)
                wexp = gp.tile([128, C], f32, tag="wexp")
                se = gp.tile([128, 1], f32, tag="se")
                nc.scalar.activation(wexp[:], psc, AF.Exp,
                                     accum_out=se[:, :1])
                rse = gp.tile([128, 1], f32, tag="rse")
                nc.vector.reciprocal(rse[:], se[:])
                wsb = gp.tile([128, C], f32, tag="wsb")
                nc.vector.tensor_scalar_mul(wsb[:], wexp[:], rse[:, :1])
                pst = psum_t()[0:C, :]
                nc.tensor.transpose(out=pst, in_=wsb[:],
                                    identity=ident[:])
                return pst

            def dense_pair(bs):
                """Dense phase for a pair of samples, activation-batched.
                All gathers issue upfront so the Pool engine's serial
                descriptor generation fully overlaps the compute chains."""
                gath = {}
                for b in bs:
                    it = load_idx(b)
                    for j in range(7):
                        gath[(b, j)] = gather_chunk(
                            it, j, q_embed,
                            dst=grp.tile([128, D], f32,
                                         tag=f"graw{b % 2}_{j}",
                                         name="graw"))
                    gath[(b, "q")] = gather_chunk(it, 7, q_embed,
                                                  dst=q_raw[b])
                    gather_chunk(it, 9, q_embed, dst=le_raw[b])
                    gath[(b, "qa")] = gather_chunk(it, 8, qa_embed)

                xts = {}
                for b in bs:
                    wT3 = wT_sb[b][:].rearrange("c (t k) -> c t k", k=17)
                    for j in range(7):
                        t0 = 8 * j
                        tcnt = min(8, S - t0)
                        xT = xT_of(gath[(b, j)], f"xT{b % 2}_{j}")
                        xts[(b, j)] = xT
                        pst = corr_T(xT)
                        nc.vector.tensor_copy(
                            wT3[:, t0:t0 + tcnt, 0:16],
                            pst[:, 0:tcnt * 16].rearrange("c (t k) -> c t k",
                                                          k=16))
                    xTq = xT_of(gath[(b, "q")], f"xTq{b % 2}")
                    pstq = corr_T(xTq)
                    qtmp = gp.tile([C, S], f32, tag="qtmp")
                    nc.vector.tensor_copy(qtmp[:], pstq[:, 0:S])
                    nc.gpsimd.tensor_copy(wT3[:, 0:S, 16], qtmp[:])
                    xts[(b, "qa")] = xT_of(gath[(b, "qa")], f"xTqa{b % 2}")
                    nc.sync.dma_start(
                        w_dram[b][:, :, :].rearrange("h c s -> c h s"),
                        wT_sb[b][:].rearrange("c (h s) -> c h s", s=CH))

                # gates: raw matmul, then sigma/tanh applied AT the
                # transposed-psum drain with per-partition bias
                gss = {}
                for b in bs:
                    for j in ["qa"] + list(range(7)):
                        psg = psum_t()
                        nc.tensor.matmul(psg[:, 0:2 * D], lhsT=xts[(b, j)][:],
                                         rhs=Wea_sb[:], start=True, stop=True)
                        gs = gsp.tile([128, 2 * D], f32, tag=f"gs{b % 2}_{j}")
                        nc.vector.tensor_copy(gs[:], psg[:, 0:2 * D])
                        gss[(b, j)] = gs

                def ea_view(b):
                    pr, half = b // 2, 64 * (b % 2)
                    E3 = E[pr][half:half + D, :].rearrange(
                        "p (t k) -> p t k", k=17)
                    A3 = A[pr][half:half + D, :].rearrange(
                        "p (t k) -> p t k", k=17)
                    return E3, A3

                for b in bs:                      # sigmoid-at-drain pass
                    E3, _ = ea_view(b)
                    for j in ["qa"] + list(range(7)):
                        pst = psum_t()
                        nc.tensor.transpose(out=pst[:], in_=gss[(b, j)][:],
                                            identity=ident[:])
                        if j == "qa":
                            etmp = gp.tile([D, S], f32, tag="etmp")
                            nc.scalar.activation(etmp[:], pst[0:D, 0:S],
                                                 AF.Sigmoid,
                                                 bias=bias_ea[0:D, 0:1])
                            nc.gpsimd.tensor_copy(E3[:, 0:S, 16], etmp[:])
                        else:
                            t0 = 8 * j
                            tcnt = min(8, S - t0)
                            pst3 = pst[:].rearrange("p (t k) -> p t k", k=16)
                            nc.scalar.activation(E3[:, t0:t0 + tcnt, 0:16],
                                                 pst3[0:D, 0:tcnt, :],
                                                 AF.Sigmoid,
                                                 bias=bias_ea[0:D, 0:1])
                for b in bs:                      # tanh-at-drain pass
                    _, A3 = ea_view(b)
                    for j in ["qa"] + list(range(7)):
                        pst = psum_t()
                        nc.tensor.transpose(out=pst[:], in_=gss[(b, j)][:],
                                            identity=ident[:])
                        if j == "qa":
                            atmp = gp.tile([D, S], f32, tag="atmp")
                            nc.scalar.activation(atmp[:], pst[D:2 * D, 0:S],
                                                 AF.Tanh,
                                                 bias=bias_ea[D:2 * D, 0:1])
                            nc.gpsimd.tensor_copy(A3[:, 0:S, 16], atmp[:])
                        else:
                            t0 = 8 * j
                            tcnt = min(8, S - t0)
                            pst3 = pst[:].rearrange("p (t k) -> p t k", k=16)
                            nc.scalar.activation(A3[:, t0:t0 + tcnt, 0:16],
                                                 pst3[D:2 * D, 0:tcnt, :],
                                                 AF.Tanh,
                                                 bias=bias_ea[D:2 * D, 0:1])

            # ---------------- scan phase ----------------
            def scan_prep(pr, ch):
                s0 = CH * ch
                wb = cw.tile([128, C * CH], f16, tag="wb")
                wb3 = wb[:].rearrange("p (c s) -> p c s", s=CH)
                for bi, b in enumerate((2 * pr, 2 * pr + 1)):
                    dst3 = wb[D * bi:D * bi + D, :].rearrange(
                        "p (c s) -> p c s", s=CH)
                    srcb = w_dram[b][ch][None, :, :].to_broadcast([D, C, CH])
                    nc.sync.dma_start(dst3[:, :, :], srcb[:, :, :])
                u = cuv.tile([128, C * (CH + 1)], f16, tag="u")
                v = cuv.tile([128, C * (CH + 1)], f16, tag="v")
                u3 = u[:].rearrange("p (c s) -> p c s", s=CH + 1)
                v3 = v[:].rearrange("p (c s) -> p c s", s=CH + 1)
                Eb = E[pr][:, s0:s0 + CH][:, None, :].to_broadcast([128, C, CH])
                Ab = A[pr][:, s0:s0 + CH][:, None, :].to_broadcast([128, C, CH])
                half = C // 2
                nc.vector.tensor_tensor(u3[:, 0:half, 1:], wb3[:, 0:half, :],
                                        Eb[:, 0:half, :], op=OP.mult)
                nc.vector.tensor_tensor(u3[:, half:, 1:], wb3[:, half:, :],
                                        Eb[:, half:, :], op=OP.mult)
                nc.scalar.activation(u3[:, :, 1:], u3[:, :, 1:], AF.Copy,
                                     bias=1.0, scale=-1.0)
                nc.gpsimd.memset(u3[:, :, 0:1], 0.0)
                nc.vector.tensor_tensor(v3[:, 0:half, 1:], wb3[:, 0:half, :],
                                        Ab[:, 0:half, :], op=OP.mult)
                nc.vector.tensor_tensor(v3[:, half:, 1:], wb3[:, half:, :],
                                        Ab[:, half:, :], op=OP.mult)
                if ch == 0:
                    nc.gpsimd.tensor_copy(v3[:, :, 0:1], M0T[:][:, :, None])
                return (wb3, u, v, v3)

            def scan_compute(pr, ch, cur, nxt):
                wb3, u, v, _ = cur
                Mt = cm.tile([128, C * (CH + 1)], f16, tag="Mt")
                Mt3 = Mt[:].rearrange("p (c s) -> p c s", s=CH + 1)
                nc.vector.tensor_tensor_scan(
                    Mt[:], u[:], v[:], 0.0, op0=OP.mult, op1=OP.add)
                if nxt is not None:
                    nc.gpsimd.tensor_copy(nxt[3][:, :, 0:1],
                                          Mt3[:, :, CH:CH + 1])
                scr = cm.tile([128, C * 16], f16, tag="scr")
                scr3 = scr[:].rearrange("p (c k) -> p c k", k=16)
                scr2 = cm.tile([128, C], f16, tag="scr2")
                for tl in range(TC):
                    t = TC * ch + tl
                    sl = 17 * tl
                    nc.vector.scalar_tensor_tensor(
                        out=scr3, in0=wb3[:, :, sl:sl + 16], scalar=1.0,
                        op0=OP.mult, in1=Mt3[:, :, sl:sl + 16], op1=OP.mult,
                        accum_out=lr[pr][:, t:t + 1])
                    nc.vector.scalar_tensor_tensor(
                        out=scr2[:][:, :, None],
                        in0=wb3[:, :, sl + 16:sl + 17],
                        scalar=1.0, op0=OP.mult, in1=Mt3[:, :, sl:sl + 1],
                        op1=OP.mult, accum_out=qr[pr][:, t:t + 1])

            def scan_pr(pr):
                chunks = [scan_prep(pr, 0), scan_prep(pr, 1)]
                for ch in range(SC):
                    nxt = chunks[ch + 1] if ch + 1 < SC else None
                    scan_compute(pr, ch, chunks[ch], nxt)
                    if ch + 2 < SC:
                        chunks.append(scan_prep(pr, ch + 2))

            # ---------------- readout: mastery -> LN -> MLP ----------------
            def readout_pr(pr):
                rows = 2 * S  # 100 rows: (b within pair, t)
                ms = pp.tile([S, 2 * H4], f32, tag=f"ms{pr}")
                for which, tsrc in ((0, qr[pr]), (2, lr[pr])):
                    pst = psum_t()[0:S, :]
                    nc.tensor.transpose(out=pst, in_=tsrc[:],
                                        identity=ident[:])
                    for bh in range(2):
                        nc.vector.tensor_copy(
                            ms[:, bh * H4 + which * D:bh * H4 + (which + 1) * D],
                            pst[:, bh * D:(bh + 1) * D])
                for bh in range(2):
                    b = 2 * pr + bh
                    nc.vector.tensor_copy(ms[:, bh * H4 + D:bh * H4 + 2 * D],
                                          q_raw[b][0:S, :])
                    nc.vector.tensor_copy(ms[:, bh * H4 + 3 * D:bh * H4 + 4 * D],
                                          le_raw[b][0:S, :])
                ms3 = ms[:].rearrange("p (b f) -> p b f", f=H4)
                mean = pp.tile([S, 2], f32, tag=f"mean{pr}")
                nc.vector.tensor_reduce(mean[:], ms3, axis=AX.X, op=OP.add)
                nc.vector.tensor_scalar_mul(mean[:], mean[:], 1.0 / H4)
                mb = mean[:][:, :, None].to_broadcast([S, 2, H4])
                nc.vector.tensor_tensor(ms3, ms3, mb, op=OP.subtract)
                sq = pp.tile([S, 2 * H4], f32, tag=f"sq{pr}")
                nc.vector.tensor_tensor(sq[:], ms[:], ms[:], op=OP.mult)
                var = pp.tile([S, 2], f32, tag=f"var{pr}")
                nc.vector.tensor_reduce(
                    var[:], sq[:].rearrange("p (b f) -> p b f", f=H4),
                    axis=AX.X, op=OP.add)
                nc.vector.tensor_scalar(var[:], var[:], 1.0 / H4, EPS,
                                        op0=OP.mult, op1=OP.add)
                sd = pp.tile([S, 2], f32, tag=f"sd{pr}")
                nc.scalar.activation(sd[:], var[:], AF.Sqrt)
                rsd = pp.tile([S, 2], f32, tag=f"rsd{pr}")
                nc.vector.reciprocal(rsd[:], sd[:])
                nc.vector.tensor_tensor(
                    ms3, ms3, rsd[:][:, :, None].to_broadcast([S, 2, H4]),
                    op=OP.mult)
                gmb = gb_rep[:, 0:H4][:, None, :].to_broadcast([S, 2, H4])
                btb = gb_rep[:, H4:2 * H4][:, None, :].to_broadcast([S, 2, H4])
                nc.vector.tensor_tensor(ms3, ms3, gmb, op=OP.mult)
                nc.vector.tensor_tensor(ms3, ms3, btb, op=OP.add)
                msT_lo = pp.tile([128, rows], f32, tag=f"msTlo{pr}")
                msT_hi = pp.tile([128, rows], f32, tag=f"msThi{pr}")
                for bh in range(2):
                    b = 2 * pr + bh
                    for fh, dstT in ((0, msT_lo), (1, msT_hi)):
                        pst = psum_t()[:, 0:S]
                        nc.tensor.transpose(
                            out=pst,
                            in_=ms[:, bh * H4 + fh * 128:bh * H4 + (fh + 1) * 128],
                            identity=ident[0:S, 0:S])
                        nc.vector.tensor_copy(dstT[:, bh * S:(bh + 1) * S],
                                              pst)

                ph = psp2.tile([rows, H4], f32, space="PSUM", tag="mlp")
                nc.tensor.matmul(ph[:], lhsT=msT_lo[:], rhs=W01[:, 0, :],
                                 start=True, stop=False)
                nc.tensor.matmul(ph[:], lhsT=msT_hi[:], rhs=W01[:, 1, :],
                                 start=False, stop=True)
                h1 = pp.tile([rows, H4], f32, tag=f"h1_{pr}")
                nc.vector.tensor_tensor(h1[:], ph[:], bias_rep[0:rows, 0:H4],
                                        op=OP.add)
                nc.scalar.activation(h1[:], h1[:], AF.Relu)
                h1T = [pp.tile([128, rows], f32, tag=f"h1T{fh}_{pr}", name=f"h1T{fh}_{pr}")
                       for fh in range(2)]
                for fh in range(2):
                    pst = psum_t()[:, 0:rows]
                    nc.tensor.transpose(out=pst,
                                        in_=h1[:, fh * 128:(fh + 1) * 128],
                                        identity=ident[0:rows, 0:rows])
                    nc.vector.tensor_copy(h1T[fh][:], pst)
                ph2 = psp2.tile([rows, H4], f32, space="PSUM", tag="mlp")
                nc.tensor.matmul(ph2[:], lhsT=h1T[0][:], rhs=W01[:, 2, :],
                                 start=True, stop=False)
                nc.tensor.matmul(ph2[:], lhsT=h1T[1][:], rhs=W01[:, 3, :],
                                 start=False, stop=True)
                h2 = pp.tile([rows, H4], f32, tag=f"h2_{pr}")
                nc.vector.tensor_tensor(h2[:], ph2[:],
                                        bias_rep[0:rows, H4:2 * H4], op=OP.add)
                scr4 = pp.tile([rows, H4], f32, tag=f"scr4_{pr}")
                logit = pp.tile([rows, 1], f32, tag=f"logit{pr}")
                nc.vector.scalar_tensor_tensor(
                    out=scr4[:], in0=h2[:], scalar=1.0, op0=OP.mult,
                    in1=Wout_rep[0:rows, :], op1=OP.mult,
                    accum_out=logit[:, 0:1])
                psig = pp.tile([rows, 1], f32, tag=f"psig{pr}")
                nc.scalar.activation(psig[:], logit[:], AF.Sigmoid,
                                     bias=bout_rep[0:rows, 0:1], scale=1.0)
                nc.sync.dma_start(
                    preds[2 * pr:2 * pr + 2, :].rearrange("b t -> (b t)")[:, None],
                    psig[:, 0:1])

            def load_late_weights():
                nc.sync.dma_start(W01[:, 0, :], W0[0:128, :])
                nc.sync.dma_start(W01[:, 1, :], W0[128:256, :])
                nc.sync.dma_start(W01[:, 2, :], W1[0:128, :])
                nc.sync.dma_start(W01[:, 3, :], W1[128:256, :])
                nc.sync.dma_start(Wout_rep[:],
                                  Wout[None, :].to_broadcast([128, H4]))
                nc.sync.dma_start(
                    bias_rep[:],
                    biases[None, :].to_broadcast([128, 2 * H4 + 2 * D]))
                nc.sync.dma_start(
                    gb_rep[:], gamma_beta[None, :].to_broadcast([S, 2 * H4]))
                nc.sync.dma_start(bout_rep[:],
                                  b_out[None, :].to_broadcast([128, 1]))
                for bb in range(2):
                    nc.sync.dma_start(M0Tf[D * bb:D * bb + D, :],
                                      M0.rearrange("c d -> d c"))
                nc.vector.tensor_copy(M0T[:], M0Tf[:])

            # ---------------- program ----------------
            dense_pair((0, 1))
            load_late_weights()
            dense_pair((2, 3))
            scan_pr(0)
            readout_pr(0)
            scan_pr(1)
            readout_pr(1)

    nc.compile()
    return nc


def _host_prepare(inputs):
    q_data = np.asarray(inputs["q_data"]).astype(np.int32)
    qa_data = np.asarray(inputs["qa_data"]).astype(np.int32)
    l_data = np.asarray(inputs["l_data"]).astype(np.int32)
    f = lambda k: np.ascontiguousarray(np.asarray(inputs[k]), dtype=np.float32)
    q_embed, qa_embed = f("q_embed"), f("qa_embed")
    key, M0 = f("key_matrix"), f("M0")
    W_ea = np.concatenate([f("W_e"), f("W_a")], axis=1)
    biases = np.concatenate([f("b0"), f("b1"), f("b_e"), f("b_a")])
    gamma_beta = np.concatenate([f("ln_gamma"), f("ln_beta")])
    W0, W1 = f("W0"), f("W1")
    Wout = f("W_out").reshape(-1)
    b_out = f("b_out").reshape(-1)

    in_maps = []
    for core in range(NCORES):
        bs = slice(core * BL, (core + 1) * BL)
        idx = np.zeros((BL, 10, QPAD), np.int32)
        idx[:, 0:7, :].reshape(BL, LPAD)[:, :LROWS] = \
            l_data[bs].reshape(BL, LROWS)
        idx[:, 7, :S] = q_data[bs]
        idx[:, 8, :S] = qa_data[bs]
        idx[:, 9, :S] = l_data[bs][:, :, L - 1]
        in_maps.append(dict(
            idx_all=np.ascontiguousarray(idx),
            q_embed=q_embed, qa_embed=qa_embed, key=key, M0=M0,
            W_ea=W_ea, W0=W0, W1=W1, Wout=Wout, biases=biases,
            gamma_beta=gamma_beta, b_out=b_out,
        ))
    return in_maps


def kernel(**inputs):
    global _BUILT
    if _BUILT is None:
        _BUILT = _build()
    nc = _BUILT
    from concourse import bass_utils
    in_maps = _host_prepare(inputs)
    res = bass_utils.run_bass_kernel_spmd(
        nc, in_maps, core_ids=list(range(NCORES)),
        trace=bool(int(os.environ.get("KERNEL_TRACE", "0"))))
    out = np.concatenate([r["preds"] for r in res.results], axis=0)
    kernel.last_results = res
    return out
